# revision 7
# baseline (speedup 1.0000x reference)
"""Trainium2 Bass kernel for nn_ItemEncoder.

Computation:
    h_type = emb[item_type]                      # [bs, na, ni, 32]
    h = concat([h_type, item], -1)               # [bs, na, ni, 43]
    z = h @ W + b                                # [bs, na, ni, 128]
    out = max_{ni} relu(z)                       # [bs, na, 128]

Device strategy (pure data parallel over bs, 4 batches/core):
  - Embedding gather + bias folded into the matmul: the host packs
    rhs columns [x_tok ; onehot(t_tok)] (K = 11 + 18 = 29, padded to
    32) and lhsT [W2 ; emb @ W[:32] + b].
  - 4-way PE row tiling: per 2048-token chunk, 4 concurrent K=32 N=512
    matmuls at tile_position (32i, 0), each into its own PSUM bank
    (rhs strip i lives on SBUF partitions 32i..32i+31).
  - PSUM drain split across both PSUM-capable engines:
      DVE    : reduce_max of the first DV groups/chunk straight from PSUM
      ScalarE: copies the other GA groups/chunk to SBUF bf16 (2x mode)
    The bf16 copies are reduced by a pairwise tensor_max fold tree at
    DVE 2x mode, batched over GPB chunks to amortize per-op bubbles.
  - Final ReLU once on the [128, 512] result (relu commutes with max).
  - ob column layout is [DVE-region | tree-region]; host unpermutes.
"""

import sys

sys.path.insert(0, "/opt/trn_rl_repo")

import ml_dtypes
import numpy as np

import concourse.bass as bass
import concourse.tile as tile
from concourse import bacc, mybir
from concourse import bass_utils

BS, NA, NI, F, H = 32, 128, 128, 11, 128
NTYPE, KEMB = 18, 32
NCORES = 8
BPC = BS // NCORES          # batches per core = 4
G = BPC * NA                # (b, na) groups per core = 512
TOK = G * NI                # tokens per core = 65536
K = F + NTYPE               # contraction dim = 29
KP = 32
CHUNK = 2048                # tokens per chunk (4 psum banks)
NCHUNK = TOK // CHUNK       # 32
GPC = CHUNK // NI           # groups per chunk = 16
F32 = mybir.dt.float32
BF16 = mybir.dt.bfloat16

DV = 0                      # groups per chunk reduced directly by DVE
GPB = 4                     # chunks per fold-tree batch

_cache = {}


def _build_program(repeat=1, dv=DV, gpb=GPB):
    key = ("nc", repeat, dv, gpb)
    if key in _cache:
        return _cache[key]

    nc = bacc.Bacc(
        "TRN2",
        target_bir_lowering=False,
        debug=False,
        enable_asserts=False,
        num_devices=NCORES,
    )

    NB = NCHUNK // 4            # DMA blocks: 512 KiB each (4 chunks)
    rhs_d = nc.dram_tensor("rhs", [NB, 128, 2048], BF16, kind="ExternalInput").ap()
    lhsT_d = nc.dram_tensor("lhsT", [128, H], BF16, kind="ExternalInput").ap()
    out_d = nc.dram_tensor("out", [H, G], F32, kind="ExternalOutput").ap()

    GA = GPC - dv               # groups per chunk drained via ScalarE

    with tile.TileContext(nc) as tc:
        with (
            tc.tile_pool(name="const", bufs=1) as const_pool,
            tc.tile_pool(name="rhs", bufs=3) as rhs_pool,
            tc.tile_pool(name="ps", bufs=2, space=bass.MemorySpace.PSUM) as ps_pool,
            tc.tile_pool(name="cp", bufs=2) as cp_pool,
            tc.tile_pool(name="s", bufs=2) as s_pool,
            tc.tile_pool(name="res", bufs=2) as res_pool,
        ):
            lt = const_pool.tile([128, H], BF16)
            nc.sync.dma_start(lt[:], lhsT_d[:])

            def body():
                ob = res_pool.tile([H, G], F32)
                orelu = res_pool.tile([H, G], F32)

                def tree(j0, cp):
                    # fold [128, gpb*GA, 128] bf16 -> ob[:, tree region]
                    ng = gpb * GA
                    cpv = cp[:].rearrange("p (g i) -> p g i", i=128)
                    s1 = s_pool.tile([128, ng * 64], BF16)
                    s1v = s1[:].rearrange("p (g i) -> p g i", i=64)
                    nc.vector.tensor_max(s1v, cpv[:, :, 0:64], cpv[:, :, 64:128])
                    s2 = s_pool.tile([128, ng * 32], BF16)
                    s2v = s2[:].rearrange("p (g i) -> p g i", i=32)
                    nc.vector.tensor_max(s2v, s1v[:, :, 0:32], s1v[:, :, 32:64])
                    s3 = s_pool.tile([128, ng * 16], BF16)
                    s3v = s3[:].rearrange("p (g i) -> p g i", i=16)
                    nc.vector.tensor_max(s3v, s2v[:, :, 0:16], s2v[:, :, 16:32])
                    s4 = s_pool.tile([128, ng * 8], BF16)
                    s4v = s4[:].rearrange("p (g i) -> p g i", i=8)
                    nc.vector.tensor_max(s4v, s3v[:, :, 0:8], s3v[:, :, 8:16])
                    base = NCHUNK * dv + j0 * GA
                    nc.vector.reduce_max(
                        ob[:, base:base + ng], s4v, axis=mybir.AxisListType.X,
                    )

                cp = None
                pending = None
                rblk = None
                for j in range(NCHUNK):
                    if j % 4 == 0:
                        blk = j // 4
                        rblk = rhs_pool.tile([128, 2048], BF16)
                        # alternate the two HWDGE rings to overlap
                        # completion latencies
                        eng = nc.sync if blk % 2 == 0 else nc.scalar
                        eng.dma_start(rblk[:], rhs_d[blk])
                    c = j % 4

                    p = ps_pool.tile([H, CHUNK], F32)
                    for i in range(4):
                        nc.tensor.matmul(
                            p[:, 512 * i:512 * (i + 1)],
                            lt[32 * i:32 * i + 32, :],
                            rblk[32 * i:32 * i + 32, c * 512:(c + 1) * 512],
                            start=True, stop=True,
                            tile_position=(32 * i, 0),
                        )

                    if dv:
                        nc.vector.reduce_max(
                            ob[:, j * dv:(j + 1) * dv],
                            p[:, 0:dv * NI].rearrange("p (g i) -> p g i", i=NI),
                            axis=mybir.AxisListType.X,
                        )
                    if GA:
                        jb = j % gpb
                        if jb == 0:
                            cp = cp_pool.tile([128, gpb * GA * 128], BF16)
                        nc.scalar.activation(
                            cp[:, jb * GA * 128:(jb + 1) * GA * 128],
                            p[:, dv * NI:],
                            mybir.ActivationFunctionType.Copy,
                        )
                        if pending is not None:
                            tree(*pending)
                            pending = None
                        if jb == gpb - 1:
                            pending = (j - gpb + 1, cp)
                if pending is not None:
                    tree(*pending)

                nc.scalar.activation(
                    orelu[:], ob[:], mybir.ActivationFunctionType.Relu
                )
                nc.sync.dma_start(out_d[:], orelu[:])

            if repeat == 1:
                body()
            else:
                with tc.For_i(0, repeat, 1):
                    body()

    nc.compile()
    _cache[key] = nc
    return nc


def _pack_inputs(item_type, item, emb, W, b):
    T_tab = (emb.astype(np.float32) @ W[:KEMB].astype(np.float32)
             + b.astype(np.float32))                       # (18, 128)
    lhsT = np.concatenate(
        [W[KEMB:].astype(np.float32), T_tab], axis=0
    ).astype(ml_dtypes.bfloat16)                           # (29, 128)
    lhsT = np.concatenate(
        [lhsT, np.zeros((KP - K, H), dtype=ml_dtypes.bfloat16)], axis=0
    )                                                      # (32, 128)
    lhsT4 = np.tile(lhsT, (4, 1))                          # (128, 128)
    eye = np.eye(NTYPE, dtype=ml_dtypes.bfloat16)

    in_maps = []
    for c in range(NCORES):
        x = item[c * BPC:(c + 1) * BPC].astype(np.float32).reshape(TOK, F)
        t = np.asarray(item_type[c * BPC:(c + 1) * BPC]).reshape(TOK)
        rhs = np.zeros((KP, TOK), dtype=ml_dtypes.bfloat16)
        rhs[:F] = x.T.astype(ml_dtypes.bfloat16)
        rhs[F:K] = eye[t].T                                 # one-hot rows
        # [KP, TOK] -> [block(8), strip(4)*KP, chunk-in-block(4)*512]
        rhs = np.ascontiguousarray(
            rhs.reshape(KP, NCHUNK // 4, 4, 4, 512)         # kp, blk, c, strip, col
            .transpose(1, 3, 0, 2, 4)                       # blk, strip, kp, c, col
            .reshape(NCHUNK // 4, 128, 2048)
        )
        in_maps.append({"rhs": rhs, "lhsT": lhsT4})
    return in_maps


def _run(in_maps, trace=False, repeat=1, dv=DV, gpb=GPB):
    nc = _build_program(repeat, dv, gpb)
    return bass_utils.run_bass_kernel_spmd(
        nc, in_maps, core_ids=list(range(NCORES)), trace=trace
    )


def _col_perm(dv=DV):
    # device ob column for global group j*GPC+k
    ga = GPC - dv
    perm = np.empty(G, dtype=np.int64)
    for j in range(NCHUNK):
        for k in range(GPC):
            perm[j * GPC + k] = (
                j * dv + k if k < dv else NCHUNK * dv + j * ga + (k - dv)
            )
    return perm


def kernel(item_type, item, emb, W, b):
    in_maps = _pack_inputs(item_type, item, emb, W, b)
    res = _run(in_maps, trace=False)
    perm = _col_perm(DV)
    out = np.empty((BS, NA, H), dtype=np.float32)
    for c in range(NCORES):
        o = res.results[c]["out"]                           # (128, 512) [h, col]
        out[c * BPC:(c + 1) * BPC] = o[:, perm].T.reshape(BPC, NA, H)
    return out
